# revision 30
# baseline (speedup 1.0000x reference)
"""AGNN (3-layer) Trainium2 kernel.

Strategy (see spec sharding_hint): nodes are partitioned across the 8
NeuronCores by destination (graph/data parallel). Edges are routed to the
core owning the destination node. Per core, destination nodes are grouped
into degree classes (in-degree padded to a multiple of 4, capped at 128);
each 128-slot "block" holds m = 128//K nodes' padded edge lists stacked on
partitions.

Device pipeline per layer:
  - xsl (fp16): gathered UNNORMALIZED source features x[src] per edge slot,
    slot-major, 33 columns per block: 32 feature dims plus a constant 1.0.
  - Cosine logits take one of two routes, selected per subrun (TF8/8 of
    subruns on the "PE route", the rest on the "DVE route"):
      * PE route: the host also ships xnt (fp8) -- normalized source
        features TRANSPOSED, 4 blocks stacked per 128 partitions -- and
        xdt (fp8), per-dst-node normalized features replicated on 4
        partition groups. One tiny matmul per block contracts over the 32
        dims: out[slot, j] = xn_src[slot] . beta*xn_dst[j]; exp runs on
        the scalar engine straight from PSUM.
      * DVE route: indicator matmul expands dst features to slots (PE);
        elementwise mult on DVE/Pool; log-tree reduce (fp16 2x); scale by
        the shipped 1/||x_src||; exp on the scalar engine.
  - Padding slots are built so their logit is about -12 (exp ~ 0): no mask
    stream needed. Garbage pair-columns of the PE route are zeroed by the
    indicator mask during weight expansion (fp16 2x on DVE).
  - Softmax normalization happens ON THE HOST: the aggregation matmul's
    stationary operand includes the constant-1 column, so its 33rd output
    row is the per-node segment sum of exp(logit). The host divides.
  - Aggregation: per-block matmul out[d, j] = sum_s xs33[s, d] * wind[s, j]
    with wind = indicator * exp(logit). Output is D-major [33, OUTW],
    copied PSUM->SBUF on the scalar engine and DMA'd out.

The source-feature gather runs on the host between layers (no fast
data-dependent gather primitive on this hardware; indirect DMA measured
~265ns/row).
"""

import math
import os
import numpy as np
import ml_dtypes
from contextlib import ExitStack

F8 = ml_dtypes.float8_e4m3fn

N_NODES = 100000
D = 32
PAY = D                            # slot payload: x dims
N_CORES = 8
NPC = N_NODES // N_CORES          # nodes per core
EPS = 1e-12
SUBRUN = 116                       # max blocks per subrun (multiple of 4)
CQ = 16                            # blocks per alpha-mult chunk (DVE route)
PAD_LOGIT = 12.0                   # padding slots get logit ~ -PAD_LOGIT
PAD_T = 8.0                        # PE-route pad logit (fp16 exp overflow cap)
POOL_MULT_OF8 = 5                  # of every 8 DVE-route subruns: mults on Pool
TF8 = 8                            # of every 8 subruns, this many on PE route
_NEFF_CACHE = {}


# ----------------------------------------------------------------------------
# host-side graph preprocessing (layer-invariant)
# ----------------------------------------------------------------------------

class Plan:
    pass


def build_plan(src, dst):
    """src/dst: int64 [E_tot] edge endpoints including self loops."""
    deg = np.bincount(dst, minlength=N_NODES)
    assert deg.max() <= 128, f"max in-degree {deg.max()} > 128 unsupported"
    K_of_node = 4 * np.ceil(deg / 4).astype(np.int64)
    K_of_node = np.maximum(K_of_node, 4)

    plan = Plan()
    plan.core_nodes = []        # per core: original node ids in sorted order
    plan.core_Ks = []
    for c in range(N_CORES):
        nodes = np.arange(c * NPC, (c + 1) * NPC)
        order = np.argsort(-K_of_node[nodes], kind="stable")
        plan.core_nodes.append(nodes[order])
        plan.core_Ks.append(K_of_node[nodes[order]])

    # class structure equalized across cores
    all_K = sorted(set(int(k) for c in range(N_CORES) for k in plan.core_Ks[c]),
                   reverse=True)
    plan.classes = []           # list of (K, m, nblk)
    for K in all_K:
        m = 128 // K
        nblk = 0
        for c in range(N_CORES):
            nk = int((plan.core_Ks[c] == K).sum())
            nblk = max(nblk, (nk + m - 1) // m)
        plan.classes.append((K, m, nblk))
    plan.NBLK = sum(nblk for _, _, nblk in plan.classes)
    plan.ARRW = plan.NBLK * D

    plan.class_arr_off = []
    off = 0
    for (K, m, nblk) in plan.classes:
        plan.class_arr_off.append(off)
        off += nblk * D
    plan.class_blk_off = []
    off = 0
    for (K, m, nblk) in plan.classes:
        plan.class_blk_off.append(off)
        off += nblk
    # output layout: per class, 2*m*ceil(nblk/2) columns (node j of block b
    # at out_off + b*m + j, nblk padded up to a multiple of 2); blocks are
    # processed in PAIRS: block 2p lands on partitions 0:32, block 2p+1 on
    # 32:64, per-column segment sums on partition 64
    plan.class_out_off = []
    off = 0
    for (K, m, nblk) in plan.classes:
        plan.class_out_off.append(off)
        off += ((nblk + 1) // 2) * 2 * m
    plan.OUTW = off
    # transposed-stream group layout: per class, ceil(nblk/2) groups of 2
    # blocks stacked on 64 partitions, 128 columns per group (matmul lhsT
    # base partition must be 0/32/64, so at most 2 lanes of 32)
    plan.class_grp_off = []
    off = 0
    for (K, m, nblk) in plan.classes:
        plan.class_grp_off.append(off)
        off += (nblk + 1) // 2
    plan.NGRP = off

    # edges grouped by dst: CSR over sorted nodes
    e_order = np.argsort(dst, kind="stable")
    src_by_dst = src[e_order]
    row_start = np.zeros(N_NODES + 1, dtype=np.int64)
    np.cumsum(deg, out=row_start[1:])

    plan.slot_src = np.zeros((N_CORES, 128, plan.NBLK), dtype=np.int32)
    plan.pad_mask = np.zeros((N_CORES, 128, plan.NBLK), dtype=bool)
    # arrangement: for core c, j, block b -> original node id (or -1)
    plan.arr_node = np.full((N_CORES, 32, plan.NBLK), -1, dtype=np.int64)
    # per-slot dst node (for the padding trick): -1 where the column is unused
    plan.slot_dst = np.full((N_CORES, 128, plan.NBLK), -1, dtype=np.int64)

    for c in range(N_CORES):
        Ks = plan.core_Ks[c]
        nodes_sorted = plan.core_nodes[c]
        pos = 0
        for ci, (K, m, nblk) in enumerate(plan.classes):
            nk = int((Ks == K).sum())
            cls_nodes = nodes_sorted[pos:pos + nk]
            pos += nk
            b0 = plan.class_blk_off[ci]
            for j_global in range(nk):
                b = j_global // m
                j = j_global % m
                node = cls_nodes[j_global]
                plan.arr_node[c, j, b0 + b] = node
                d0 = deg[node]
                p0 = j * K
                ss = src_by_dst[row_start[node]:row_start[node] + d0]
                plan.slot_src[c, p0:p0 + d0, b0 + b] = ss
                plan.slot_dst[c, p0:p0 + K, b0 + b] = node
                plan.pad_mask[c, p0 + d0:p0 + K, b0 + b] = True
    return plan


def host_normalize(x):
    nrm = np.sqrt((x.astype(np.float64) ** 2).sum(axis=1))
    nrm = np.maximum(nrm, EPS).astype(np.float32)
    xn = (x / nrm[:, None]).astype(np.float32)
    return xn, nrm


def host_layer_inputs(plan, x_full, beta):
    """Build per-core device inputs for one layer from the full node features."""
    xn, nrm = host_normalize(x_full)
    invn = (1.0 / nrm).astype(np.float32)
    ins = []
    for c in range(N_CORES):
        ss = plan.slot_src[c]                       # [128, NBLK]
        pm = plan.pad_mask[c]
        dstn = plan.slot_dst[c][pm]
        unused = (plan.slot_dst[c] < 0) & ~pm

        xg = x_full[ss].astype(np.float32)          # [128, NBLK, D]
        bn = invn[ss].astype(np.float16)            # [128, NBLK]
        # padding slots: x := -0.5 * xn_dst, invn := 2*PAD_LOGIT -> logit -12
        # (small |x| keeps the tiny residual weight from polluting the sums)
        xg[pm] = -0.5 * xn[dstn]
        bn[pm] = 2.0 * PAD_LOGIT
        xg[unused] = 0.0
        bn[unused] = 1.0

        xsl = xg.astype(np.float16)

        # normalized slot table for the transposed (PE-route) stream
        xns = xn[ss]                                # [128, NBLK, D] fp32
        xns[pm] = -PAD_T * xn[dstn]
        xns[unused] = 0.0

        xnt = np.zeros((64, plan.NGRP * 128), dtype=F8)
        xdt = np.zeros((64, plan.OUTW), dtype=np.float16)
        for ci, (K, m, nblk) in enumerate(plan.classes):
            b0 = plan.class_blk_off[ci]
            g0 = plan.class_grp_off[ci]
            G = (nblk + 1) // 2
            arr = np.zeros((128, G * 2, D), dtype=np.float32)
            arr[:, :nblk] = xns[:, b0:b0 + nblk]
            # xnt[l*32+d, (g0+g)*128 + s] = arr[s, 2g+l, d]
            t = arr.reshape(128, G, 2, D).transpose(2, 3, 1, 0)  # [2, D, G, 128]
            xnt[:, g0 * 128:(g0 + G) * 128] = \
                t.reshape(64, G * 128).astype(F8)
            # block-diagonal: xdt[(b%2)*32+d, o0 + b*m + j] = beta*xn[node][d]
            o0 = plan.class_out_off[ci]
            nodes = plan.arr_node[c, :m, b0:b0 + nblk]          # [m, nblk]
            valid = nodes >= 0
            nb2 = ((nblk + 1) // 2) * 2
            xd = np.zeros((m, nb2, D), dtype=np.float32)
            xd[:, :nblk][valid] = beta * xn[nodes[valid]]
            xdT = xd.transpose(2, 1, 0).reshape(D, nb2, m)       # [D, nb2, m]
            xdT8 = xdT.astype(np.float16)
            for l in range(2):
                xdt[l * 32:(l + 1) * 32, o0:o0 + nb2 * m] = np.where(
                    (np.arange(nb2) % 2 == l)[None, :, None], xdT8, 0
                ).reshape(D, nb2 * m)

        xarr = np.zeros((32, plan.ARRW), dtype=np.float16)
        for ci, (K, m, nblk) in enumerate(plan.classes):
            a0 = plan.class_arr_off[ci]
            b0 = plan.class_blk_off[ci]
            nodes = plan.arr_node[c, :m, b0:b0 + nblk]     # [m, nblk]
            valid = nodes >= 0
            xa = np.zeros((m, nblk, D), dtype=np.float32)
            xa[valid] = beta * xn[nodes[valid]]
            xarr[:m, a0:a0 + nblk * D] = xa.reshape(m, nblk * D).astype(np.float16)
        ins.append({
            "xsl": np.ascontiguousarray(xsl.reshape(128, plan.NBLK * PAY)),
            "bn": np.ascontiguousarray(bn),
            "xarr": xarr,
            "xnt": xnt,
            "xdt": xdt,
        })
    return ins


def host_collect_output(plan, oarrs):
    """oarrs: per-core [65, OUTW] fp16 -> full [N, D].

    Block 2p's sums sit on partitions 0:32 of its column range, block 2p+1's
    on 32:64; per-column segment sums on partition 64. Cross terms ignored."""
    out = np.zeros((N_NODES, D), dtype=np.float32)
    for c in range(N_CORES):
        oa = oarrs[c].astype(np.float32)
        for ci, (K, m, nblk) in enumerate(plan.classes):
            o0 = plan.class_out_off[ci]
            b0 = plan.class_blk_off[ci]
            nb2 = ((nblk + 1) // 2) * 2
            seg = oa[:, o0:o0 + nb2 * m].reshape(65, nb2, m)
            lane = (np.arange(nb2) % 2) * 32                # [nb2]
            vals = np.empty((D, nb2, m), dtype=np.float32)
            for l in (0, 1):
                selb = np.arange(nb2) % 2 == l
                vals[:, selb] = seg[l * 32:l * 32 + D][:, selb]
            vals /= np.maximum(seg[64], 1e-20)
            nodes = plan.arr_node[c, :m, b0:b0 + nblk]     # [m, nblk]
            valid = nodes >= 0                              # [m, nblk]
            nv = nodes.T[valid.T]                           # row-major (b, j)
            out[nv] = vals[:, :nblk].transpose(1, 2, 0)[valid.T]
    return out


# ----------------------------------------------------------------------------
# device kernel
# ----------------------------------------------------------------------------

def host_indicators(plan):
    """Packed per-class indicator matrices (identical for every core)."""
    uniq = []
    seen = set()
    for (K, m, nblk) in plan.classes:
        if (K, m) not in seen:
            seen.add((K, m))
            uniq.append((K, m))
    plan.ind_uniq = uniq
    indk = np.zeros((128, sum(m for _, m in uniq)), dtype=np.float16)
    indkt = np.zeros((32, 128 * len(uniq)), dtype=np.float16)
    plan.ind_off = {}
    off = 0
    for i, (K, m) in enumerate(uniq):
        plan.ind_off[(K, m)] = (off, i)
        p = np.arange(128)
        sel = (p // K) < m
        indk[sel, off + (p // K)[sel]] = 1.0
        indkt[:m, i * 128:(i + 1) * 128] = indk[:, off:off + m].T
        off += m
    plan.indk_w = indk.shape[1]
    plan.n_ind = len(uniq)
    return indk, indkt


def on_pe_route(si):
    return (si * TF8) % 8 < TF8


def build_nc(plan):
    import concourse.bass as bass
    import concourse.tile as tile
    from concourse import bacc, mybir

    f32 = mybir.dt.float32
    f16 = mybir.dt.float16
    f8 = mybir.dt.float8e4
    nc = bacc.Bacc("TRN2", target_bir_lowering=False, debug=False)
    xsl_d = nc.declare_dram_parameter("xsl", [128, plan.NBLK * PAY], f16, isOutput=False)
    bn_d = nc.declare_dram_parameter("bn", [128, plan.NBLK], f16, isOutput=False)
    xarr_d = nc.declare_dram_parameter("xarr", [32, plan.ARRW], f16, isOutput=False)
    xnt_d = nc.declare_dram_parameter("xnt", [64, plan.NGRP * 128], f8, isOutput=False)
    xdt_d = nc.declare_dram_parameter("xdt", [64, plan.OUTW], f16, isOutput=False)
    indk_d = nc.declare_dram_parameter("indk", [128, plan.indk_w], f16, isOutput=False)
    indkt_d = nc.declare_dram_parameter("indkt", [32, 128 * plan.n_ind], f16, isOutput=False)
    oarr_d = nc.declare_dram_parameter("oarr", [65, plan.OUTW], f16, isOutput=True)

    # subrun schedule: (class_idx, blk_off_in_class, nblk_sub)
    subruns = []
    for ci, (K, m, nblk) in enumerate(plan.classes):
        b = 0
        while b < nblk:
            n = min(SUBRUN, nblk - b)
            subruns.append((ci, b, n))
            b += n

    any_dve_route = any(not on_pe_route(si) for si in range(len(subruns)))
    any_pe_route = any(on_pe_route(si) for si in range(len(subruns)))

    with tile.TileContext(nc) as tc, ExitStack() as ctx:
        const = ctx.enter_context(tc.tile_pool(name="const", bufs=1))
        xpool = ctx.enter_context(tc.tile_pool(name="xsl", bufs=6))
        apool = ctx.enter_context(tc.tile_pool(name="arr", bufs=2))
        tpool = ctx.enter_context(tc.tile_pool(name="xnt", bufs=4))
        epool = ctx.enter_context(tc.tile_pool(name="esb", bufs=3))
        wpool = ctx.enter_context(tc.tile_pool(name="work", bufs=3))
        wipool = ctx.enter_context(tc.tile_pool(name="windp", bufs=3))
        spool = ctx.enter_context(tc.tile_pool(name="small", bufs=3))
        opool = ctx.enter_context(tc.tile_pool(name="outp", bufs=3))
        ps_l = ctx.enter_context(tc.tile_pool(name="psl", bufs=3, space="PSUM"))
        ps_x = ctx.enter_context(tc.tile_pool(name="psx", bufs=2, space="PSUM"))
        ps_a = ctx.enter_context(tc.tile_pool(name="psa", bufs=3, space="PSUM"))

        # resident constants
        indk_sb = const.tile([128, plan.indk_w], f16)
        nc.sync.dma_start(out=indk_sb[:], in_=indk_d[:])
        ones_sb = const.tile([128, 1], f16)
        nc.vector.memset(ones_sb[:], 1.0)
        if any_dve_route:
            indkt_sb = const.tile([32, 128 * plan.n_ind], f16)
            nc.sync.dma_start(out=indkt_sb[:], in_=indkt_d[:])
            bn_sb = const.tile([128, plan.NBLK], f16)
            nc.sync.dma_start(out=bn_sb[:], in_=bn_d[:])
        else:
            indkt_sb = bn_sb = None


        xdt_sb = None
        if any_pe_route:
            xdt_sb = const.tile([64, plan.OUTW], f16)
            nc.sync.dma_start(out=xdt_sb[:], in_=xdt_d[:])

        state = {}

        def ctx_of(si):
            (ci, bo, R) = subruns[si]
            K, m, nblk = plan.classes[ci]
            a0 = plan.class_arr_off[ci] + bo * D
            b0 = plan.class_blk_off[ci] + bo
            o0 = plan.class_out_off[ci] + bo * m
            g0 = plan.class_grp_off[ci] + bo // 2
            ioff, iidx = plan.ind_off[(K, m)]
            indkt = (indkt_sb[:, iidx * 128:(iidx + 1) * 128]
                     if indkt_sb is not None else None)
            return (K, m, a0, b0, o0, g0,
                    indk_sb[:, ioff:ioff + m], indkt, R)

        def emit_P(si):
            """Prefetch stage: issue the subrun's input DMAs early."""
            K, m, a0, b0, o0, g0, indk, indkt, R = ctx_of(si)
            xs = xpool.tile([128, (SUBRUN + 1) * PAY], f16, tag="xs")
            nc.sync.dma_start(out=xs[:, :R * PAY],
                              in_=xsl_d[:, b0 * PAY:(b0 + R) * PAY])
            if R % 2:
                nc.gpsimd.memset(xs[:, R * PAY:(R + 1) * PAY], 0.0)
            st = state.setdefault(si, {})
            st["xs"] = xs
            if on_pe_route(si):
                G = (R + 1) // 2
                xnt = tpool.tile([64, ((SUBRUN + 1) // 2) * 128], f8, tag="xnt")
                nc.sync.dma_start(out=xnt[:, :G * 128],
                                  in_=xnt_d[:, g0 * 128:(g0 + G) * 128])
                st["xnt"] = xnt
            else:
                xa = apool.tile([32, SUBRUN * D], f16, tag="xa")
                nc.sync.dma_start(out=xa[:m, :R * D],
                                  in_=xarr_d[:m, a0:a0 + R * D])
                st["xa"] = xa

        def emit_A_pe(si):
            """PE route: one fp8 matmul per block PAIR for the cosine logits
            (two blocks stacked on 64 partitions, block-diagonal dst operand)."""
            K, m, a0, b0, o0, g0, indk, indkt, R = ctx_of(si)
            st = state[si]
            xnt = st.pop("xnt")
            G = (R + 1) // 2
            esb = epool.tile([128, (SUBRUN + 1) * 32], f16, tag="esb")
            PC = max(1, 512 // (2 * m))    # pairs per logit psum tile
            for q in range((G + PC - 1) // PC):
                cp = min(PC, G - q * PC)
                alps = ps_l.tile([128, 512], f32, tag="alps")
                for p in range(cp):
                    pr = q * PC + p
                    nc.tensor.matmul(
                        out=alps[:, p * 2 * m:(p + 1) * 2 * m],
                        lhsT=xnt[:, pr * 128:(pr + 1) * 128],
                        rhs=xdt_sb[:, o0 + 2 * pr * m:o0 + 2 * (pr + 1) * m],
                        start=True, stop=True)
                nc.scalar.activation(
                    esb[:, q * PC * 2 * m:(q * PC + cp) * 2 * m],
                    alps[:, :cp * 2 * m],
                    mybir.ActivationFunctionType.Exp, 0.0, 1.0)
            st["esb"] = esb

        def emit_A_dve(si):
            """DVE route: expand dst features, elementwise mult (DVE/Pool)."""
            K, m, a0, b0, o0, g0, indk, indkt, R = ctx_of(si)
            st = state[si]
            xs, xa = st["xs"], st.pop("xa")
            xs3 = xs[:, :R * PAY].rearrange("p (b w) -> p b w", b=R, w=PAY)

            on_pool = (si * POOL_MULT_OF8) % 8 < POOL_MULT_OF8
            eng = nc.gpsimd if on_pool else nc.vector
            prod = wpool.tile([128, SUBRUN * D], f16, tag="prod")
            for q in range((R + CQ - 1) // CQ):
                cb = min(CQ, R - q * CQ)
                xnd = ps_x.tile([128, CQ * D], f32, tag="xnd")
                nc.tensor.matmul(out=xnd[:, :cb * D], lhsT=indkt[:m, :],
                                 rhs=xa[:m, q * CQ * D:(q * CQ + cb) * D],
                                 start=True, stop=True)
                eng.tensor_tensor(
                    out=prod[:, q * CQ * D:(q * CQ + cb) * D].rearrange(
                        "p (b w) -> p b w", b=cb, w=D),
                    in0=xs3[:, q * CQ:q * CQ + cb, 0:D],
                    in1=xnd[:, :cb * D].rearrange("p (b w) -> p b w", b=cb, w=D),
                    op=mybir.AluOpType.mult)
            st["prod"] = prod

        def emit_B_pe(si):
            """PE route: wind = indicator * exp(logit)  (fp16 2x on DVE)."""
            K, m, a0, b0, o0, g0, indk, indkt, R = ctx_of(si)
            st = state[si]
            esb = st.pop("esb")
            R2 = R + (R % 2)               # cover the padded block of a lone pair
            wind = wipool.tile([128, (SUBRUN + 1) * 32], f16, tag="wind")
            windv = wind[:, :R2 * m].rearrange("p (b j) -> p b j", b=R2, j=m)
            nc.vector.tensor_tensor(
                out=windv,
                in0=esb[:, :R2 * m].rearrange("p (b j) -> p b j", b=R2, j=m),
                in1=indk[:, None, :m].to_broadcast([128, R2, m]),
                op=mybir.AluOpType.mult)
            st["windv"] = wind

        def emit_B_dve(si):
            """DVE route: tree-reduce + 1/||x|| scale + exp + wind expand."""
            K, m, a0, b0, o0, g0, indk, indkt, R = ctx_of(si)
            st = state[si]
            pr3 = st.pop("prod")[:, :R * D].rearrange("p (b w) -> p b w", b=R, w=D)
            t16 = wpool.tile([128, SUBRUN * 16], f16, tag="t16")
            t16v = t16[:, :R * 16].rearrange("p (b w) -> p b w", b=R, w=16)
            nc.vector.tensor_tensor(out=t16v, in0=pr3[:, :, 0:16],
                                    in1=pr3[:, :, 16:32], op=mybir.AluOpType.add)
            t8 = spool.tile([128, SUBRUN * 8], f16, tag="t8")
            t8v = t8[:, :R * 8].rearrange("p (b w) -> p b w", b=R, w=8)
            nc.vector.tensor_tensor(out=t8v, in0=t16v[:, :, 0:8],
                                    in1=t16v[:, :, 8:16], op=mybir.AluOpType.add)
            t4 = spool.tile([128, SUBRUN * 4], f16, tag="t4")
            t4v = t4[:, :R * 4].rearrange("p (b w) -> p b w", b=R, w=4)
            nc.vector.tensor_tensor(out=t4v, in0=t8v[:, :, 0:4],
                                    in1=t8v[:, :, 4:8], op=mybir.AluOpType.add)
            t2 = spool.tile([128, SUBRUN * 2], f16, tag="t2")
            t2v = t2[:, :R * 2].rearrange("p (b w) -> p b w", b=R, w=2)
            nc.vector.tensor_tensor(out=t2v, in0=t4v[:, :, 0:2],
                                    in1=t4v[:, :, 2:4], op=mybir.AluOpType.add)
            t1 = spool.tile([128, SUBRUN], f16, tag="t1")
            nc.vector.tensor_tensor(out=t1[:, :R], in0=t2v[:, :, 0],
                                    in1=t2v[:, :, 1], op=mybir.AluOpType.add)
            alpha = spool.tile([128, SUBRUN], f16, tag="alpha")
            nc.vector.tensor_tensor(out=alpha[:, :R], in0=t1[:, :R],
                                    in1=bn_sb[:, b0:b0 + R], op=mybir.AluOpType.mult)
            e = spool.tile([128, SUBRUN], f16, tag="e")
            nc.scalar.activation(e[:, :R], alpha[:, :R],
                                 mybir.ActivationFunctionType.Exp, 0.0, 1.0)
            wind = wipool.tile([128, (SUBRUN + 1) * 32], f16, tag="wind")
            windv = wind[:, :R * m].rearrange("p (b j) -> p b j", b=R, j=m)
            nc.vector.tensor_tensor(
                out=windv,
                in0=indk[:, None, :m].to_broadcast([128, R, m]),
                in1=e[:, :R, None].to_broadcast([128, R, m]),
                op=mybir.AluOpType.mult)
            if R % 2:
                nc.gpsimd.memset(wind[:, R * m:(R + 1) * m], 0.0)
            st["windv"] = wind

        def emit_C(si):
            K, m, a0, b0, o0, g0, indk, indkt, R = ctx_of(si)
            st = state.pop(si)
            xs, wind = st["xs"], st["windv"]
            G = (R + 1) // 2               # block pairs (incl padded lone pair)
            OC = max(1, 512 // (2 * m))    # pairs per output psum tile
            ocp = opool.tile([65, (SUBRUN + 1) * 32], f16, tag="ocp")
            for q in range((G + OC - 1) // OC):
                cp = min(OC, G - q * OC)
                c0 = q * OC * 2 * m
                oacc = ps_a.tile([65, 512], f32, tag="oacc")
                for p in range(cp):
                    pr = q * OC + p
                    nc.tensor.matmul(
                        out=oacc[0:64, p * 2 * m:(p + 1) * 2 * m],
                        lhsT=xs[:, 2 * pr * PAY:(2 * pr + 2) * PAY],
                        rhs=wind[:, 2 * pr * m:2 * (pr + 1) * m],
                        start=True, stop=True)
                # per-column segment sums on partition 64
                nc.tensor.matmul(out=oacc[64:65, :cp * 2 * m],
                                 lhsT=ones_sb[:, 0:1],
                                 rhs=wind[:, c0:c0 + cp * 2 * m],
                                 start=True, stop=True)
                nc.scalar.activation(ocp[:, c0:c0 + cp * 2 * m],
                                     oacc[:, :cp * 2 * m],
                                     mybir.ActivationFunctionType.Copy, 0.0, 1.0)
            nc.scalar.dma_start(out=oarr_d[:, o0:o0 + 2 * G * m],
                                in_=ocp[:, :2 * G * m])

        n = len(subruns)
        for t in range(n + 4):
            if t < n:
                emit_P(t)
            if 2 <= t < n + 2:
                (emit_A_pe if on_pe_route(t - 2) else emit_A_dve)(t - 2)
            if 3 <= t < n + 3:
                s_ = t - 3
                (emit_B_pe if on_pe_route(s_) else emit_B_dve)(s_)
            if t >= 4:
                emit_C(t - 4)

    nc.compile()
    return nc


# ----------------------------------------------------------------------------
# entry point
# ----------------------------------------------------------------------------

def kernel(x, edge_index, beta1, beta2, beta3):
    x = np.asarray(x, dtype=np.float32)
    edge_index = np.asarray(edge_index)
    betas = [float(np.asarray(b).reshape(-1)[0]) for b in (beta1, beta2, beta3)]

    loops = np.arange(N_NODES, dtype=edge_index.dtype)
    src = np.concatenate([edge_index[0], loops]).astype(np.int64)
    dst = np.concatenate([edge_index[1], loops]).astype(np.int64)

    plan = build_plan(src, dst)
    indk, indkt = host_indicators(plan)

    from concourse.bass_utils import run_bass_kernel_spmd
    key = (plan.NBLK, tuple(plan.classes))
    if key not in _NEFF_CACHE:
        _NEFF_CACHE[key] = build_nc(plan)
    nc = _NEFF_CACHE[key]

    cur = x
    for li in range(3):
        ins = host_layer_inputs(plan, cur, betas[li])
        for m in ins:
            m["indk"] = indk
            m["indkt"] = indkt
        res = run_bass_kernel_spmd(nc, ins, core_ids=list(range(N_CORES)))
        oarrs = [res.results[c]["oarr"] for c in range(N_CORES)]
        cur = host_collect_output(plan, oarrs)
    return cur
